# revision 19
# baseline (speedup 1.0000x reference)
"""Trainium2 Bass kernel for nn_LinearPPI (block-sparse gene-gene message passing).

Computation (reference):
    out[b, 8*g_out + o] = sum_{n: block_out[n]=g_out} sum_i x[b, 8*block_in[n] + i] * w[n, i, o]
    out += x   (residual)

Strategy:
  - Blocks sorted by destination gene; destination genes sharded over 8 cores
    (edge/expert parallel, no collectives needed).
  - Per core, genes are packed into "quads" of QG (default 2) genes.  A quad
    owns a [QG*8, 128] region of a PSUM bank (QG genes x 8 outs, 128 batch),
    laid out transposed (out^T).  16 quads fill one PSUM bank tile [128, 512].
  - Work is a stream of "windows": 16 x-slabs (one slab = 8 rows of x^T for
    one source gene = [8, 128]) stacked to a [128, 128] STATIONARY lhsT, and
    a matching scattered weight tile [128, QG*8] as the MOVING rhs.  One
    matmul per window:
        psum[0:128, f0:f0+QG*8] (+)= x_win.T @ w_win   (K=128, M=128, N=QG*8)
    The output is batch-major [128, QG*8] so the moving free dim is only
    QG*8=16 rows -> ~16 PE cycles per window instead of 128 (the batch dim
    rides in the stationary operand).  PSUM banks are fully dense: 32 quads
    per [128, 512] bank, 8 banks hold the core's whole output.
    PSUM per-element has_written bits turn the writes into a correct
    segment-sum; each quad's first matmul uses start=True (the bit clear
    only touches regions that are either finished or not yet started,
    and quads in one bank execute back-to-back on the PE).
  - The whole stream (x slabs + scattered weights) is fp8 e3m4: x scaled by
    XS=2, w by WS=32 so both sit in e3m4's normal range.  PSUM accumulates in
    fp32 at scale XS*WS; the host descales.  e3m4's 4 mantissa bits keep the
    end-to-end relative error ~1.2e-2 (measured), under the 2e-2 gate, while
    halving both DMA bytes and leaving PE at 1 cycle/row.
  - The residual (+x) is applied on the host in fp32 (exact), which also
    drops the 4000 virtual identity blocks from the stream.
  - The x-slab gather is done on the host (indices are known at trace time),
    producing a sequential HBM stream -> all device DMAs are large and
    contiguous (memory-bound regime).
  - The per-core window schedule is made identical across cores (rank-sorted
    window-count maxima + zero-padding) so a single SPMD program serves all
    8 cores; per-core variation lives only in the streamed data.
  - Output is slot-ordered out^T; the host inverse-permutes, transposes and
    concatenates shards.  No all-reduce: destination sharding makes each
    core's output disjoint.
"""

import math
import ml_dtypes
import numpy as np

import concourse.bass as bass
import concourse.bacc as bacc
import concourse.mybir as mybir
from concourse.tile import TileContext
from concourse.bass_utils import run_bass_kernel_spmd


class Cfg:
    def __init__(self, G=4000, B=8, BATCH=128, NCORES=8, chunk=24, qg=2,
                 xs=2.0, ws=32.0):
        assert G % NCORES == 0
        self.G, self.B, self.BATCH, self.NCORES = G, B, BATCH, NCORES
        self.GPC = G // NCORES            # genes per core
        self.QG = qg                      # genes per quad (M = QG*B)
        assert self.GPC % self.QG == 0
        self.NQ = self.GPC // self.QG     # quads per core
        self.UQ = 16                      # quads per drain unit (half a bank)
        self.UW = self.UQ * self.QG * B   # unit width in psum columns (256)
        self.NU = math.ceil(self.NQ / self.UQ)
        self.SLOTS = 16                   # slabs per window (K = 128)
        self.CH = chunk                   # windows per DMA chunk
        self.XS, self.WS = xs, ws         # fp8 pre-scales for x and w
        self.FP8_MAX = 15.5               # e3m4 saturation
        self.stream_np = ml_dtypes.float8_e3m4
        self.stream_dt = mybir.dt.float8e3
        self.out_dt = mybir.dt.float16


def _pack_host(cfg, x, w, block_in, block_out):
    """Sort/shard/pad on the host. Returns (in_maps, w_sched, decode_quads)."""
    G, B, BATCH, NC = cfg.G, cfg.B, cfg.BATCH, cfg.NCORES

    # Residual is applied on the host; stream only the real blocks.
    src = np.asarray(block_in, dtype=np.int64)
    dst = np.asarray(block_out, dtype=np.int64)
    fmax = cfg.FP8_MAX

    order = np.argsort(dst, kind="stable")
    src_s = src[order]
    w_s = np.clip(np.asarray(w, dtype=np.float32)[order] * cfg.WS,
                  -fmax, fmax).astype(cfg.stream_np)
    w_s = np.ascontiguousarray(w_s)
    counts = np.bincount(dst, minlength=G)
    starts = np.zeros(G + 1, dtype=np.int64)
    np.cumsum(counts, out=starts[1:])

    # x^T slabs: xslab[g] = x[:, 8g:8g+8].T  -> [G, 8, BATCH]
    xslab = np.clip(np.asarray(x, dtype=np.float32).T * cfg.XS,
                    -fmax, fmax).reshape(G, B, BATCH).astype(cfg.stream_np)
    xslab = np.ascontiguousarray(xslab)

    # --- balanced gene->core assignment (snake over count-sorted genes) ---
    order_g = np.argsort(-counts, kind="stable")
    core_of = np.empty(G, dtype=np.int64)
    for r in range(0, G, 2 * NC):
        blk = order_g[r : r + 2 * NC]
        pat = list(range(NC)) + list(range(NC - 1, -1, -1))
        for i, g in enumerate(blk):
            core_of[g] = pat[i]

    # --- per-core quad packing: target sums that are multiples of SLOTS ---
    per_core = []
    for c in range(NC):
        genes = np.where(core_of == c)[0]  # this core's genes
        pool = sorted(genes.tolist(), key=lambda g: -counts[g])
        quads = []
        for _ in range(cfg.NQ):
            q = [pool.pop(0)]                       # largest remaining
            while pool and len(q) < cfg.QG - 1:     # middle picks: big/small mix
                q.append(pool.pop(0) if len(q) % 2 else pool.pop(-1))
            if pool and len(q) < cfg.QG:
                s3 = sum(int(counts[g]) for g in q)
                # last pick: minimize padding to the next multiple of SLOTS
                best_i = min(range(len(pool)),
                             key=lambda i: (-(s3 + int(counts[pool[i]])))
                             % cfg.SLOTS)
                q.append(pool.pop(best_i))
            q.sort()
            quads.append(q)
        assert not pool
        q_slabs = np.array([sum(int(counts[g]) for g in q) for q in quads])
        q_wins = np.ceil(q_slabs / cfg.SLOTS).astype(np.int64)
        q_wins = np.maximum(q_wins, 1)
        rank = np.argsort(-q_wins, kind="stable")
        per_core.append(([quads[j] for j in rank], q_wins[rank]))

    # common schedule: per rank, max window count over cores
    w_sched = np.max(np.stack([pc[1] for pc in per_core]), axis=0)
    cum_w = np.zeros(cfg.NQ + 1, dtype=np.int64)
    np.cumsum(w_sched, out=cum_w[1:])
    w_tot = int(cum_w[-1])

    # --- build per-core streams -------------------------------------------
    in_maps = []
    decode_quads = []
    for c in range(NC):
        quads_r, _ = per_core[c]
        slab_gene = np.full(w_tot * cfg.SLOTS, -1, dtype=np.int64)
        blk_ids, blk_pos, blk_rel = [], [], []
        for j in range(cfg.NQ):
            base = cum_w[j] * cfg.SLOTS
            p = 0
            for r, g in enumerate(quads_r[j]):
                s0, n = int(starts[g]), int(counts[g])
                ids = np.arange(s0, s0 + n)
                blk_ids.append(ids)
                blk_pos.append(base + p + np.arange(n))
                blk_rel.append(np.full(n, r, dtype=np.int64))
                p += n
            assert p <= int(w_sched[j]) * cfg.SLOTS
        blk_ids = np.concatenate(blk_ids)
        blk_pos = np.concatenate(blk_pos)
        blk_rel = np.concatenate(blk_rel)
        slab_gene[blk_pos] = src_s[blk_ids]

        # x slabs: [W, 128, BATCH]
        xg = np.zeros((w_tot * cfg.SLOTS, B, BATCH), dtype=cfg.stream_np)
        m = slab_gene >= 0
        xg[m] = xslab[slab_gene[m]]
        xg = xg.reshape(w_tot, cfg.SLOTS * B, BATCH)

        # scattered weights: [W, 128, 32]
        wg5 = np.zeros((w_tot, cfg.SLOTS, B, cfg.QG, B), dtype=cfg.stream_np)
        wg5[blk_pos // cfg.SLOTS, blk_pos % cfg.SLOTS, :, blk_rel, :] = w_s[blk_ids]
        wg = wg5.reshape(w_tot, cfg.SLOTS * B, cfg.QG * B)

        # combined stream, chunk-major: each chunk of CH windows is one
        # contiguous [128, CH*PW] DRAM block -> every DMA is a single
        # linear ~1MB read.
        st = np.concatenate([xg, wg], axis=2)          # [W, 128, PW]
        PW = BATCH + cfg.QG * B
        n_chunks = -(-w_tot // cfg.CH)
        pad = n_chunks * cfg.CH - w_tot
        if pad:
            st = np.concatenate(
                [st, np.zeros((pad, cfg.SLOTS * B, PW), dtype=cfg.stream_np)])
        st = np.ascontiguousarray(
            st.reshape(n_chunks, cfg.CH, cfg.SLOTS * B, PW)
            .transpose(0, 2, 1, 3)).reshape(n_chunks * cfg.SLOTS * B, cfg.CH * PW)

        in_maps.append({"st": st})
        decode_quads.append(quads_r)

    return in_maps, w_sched, decode_quads


def _build_nc(cfg, w_sched):
    """Trace the (core-uniform) Bass program."""
    w_tot = int(np.sum(w_sched))
    PW = cfg.BATCH + cfg.QG * cfg.B   # stream width per window
    n_chunks = -(-w_tot // cfg.CH)
    nc = bacc.Bacc("TRN2")
    st = nc.dram_tensor("st", [n_chunks * 128, cfg.CH * PW], cfg.stream_dt,
                        kind="ExternalInput")
    out = nc.dram_tensor("out", [128, cfg.NQ * cfg.QG * cfg.B], cfg.out_dt,
                         kind="ExternalOutput")

    cum_w = np.zeros(cfg.NQ + 1, dtype=np.int64)
    np.cumsum(w_sched, out=cum_w[1:])
    CH = cfg.CH
    NW = cfg.BATCH            # rhs free width per window (128)

    with TileContext(nc) as tc:
        with (
            tc.tile_pool(name="stp", bufs=4) as stp,
            tc.tile_pool(name="psp", bufs=3, space="PSUM") as psp,
            tc.tile_pool(name="outp", bufs=2) as outp,
        ):
            RW = cfg.QG * cfg.B       # psum region width per quad
            st_t = None
            # Drain units: each owns its own psum tile, so unit copies never
            # share a tile with pending matmuls (no WAR stall).  Unit list is
            # (quad_lo, quad_hi, dma_engine); the final unit is a 2-quad
            # sliver drained via SP (cheapest HWDGE, idle at the end).
            units = []
            for u in range(cfg.NU):
                j0, j1 = u * cfg.UQ, min((u + 1) * cfg.UQ, cfg.NQ)
                if u == cfg.NU - 1 and j1 - j0 > 2:
                    units.append((j0, j1 - 2, nc.scalar))
                    units.append((j1 - 2, j1, nc.sync))
                else:
                    units.append((j0, j1, nc.scalar if u < cfg.NU - 1
                                  else nc.sync))
            for j0, j1, eng in units:
                uw = RW * (j1 - j0)
                ps = psp.tile([128, uw], mybir.dt.float32)
                ot = outp.tile([128, uw], cfg.out_dt)
                for j in range(j0, j1):
                    f0 = RW * (j - j0)
                    t_first = int(cum_w[j])
                    for t in range(int(cum_w[j]), int(cum_w[j + 1])):
                        if t % CH == 0:
                            c = t // CH
                            st_t = stp.tile([128, CH * PW], cfg.stream_dt)
                            # Pool's SWDGE has the shortest cold-start; use
                            # it for the very first chunk, SP thereafter.
                            eng_in = nc.gpsimd if c == 0 else nc.sync
                            eng_in.dma_start(
                                out=st_t[:, :],
                                in_=st[c * 128 : (c + 1) * 128, :])
                        k = t % CH
                        nc.tensor.matmul(
                            ps[:, f0 : f0 + RW],
                            st_t[:, k * PW : k * PW + NW],
                            st_t[:, k * PW + NW : (k + 1) * PW],
                            start=(t == t_first),
                            stop=(t == int(cum_w[j + 1]) - 1),
                            tile_position=(0, 0),
                        )
                nc.vector.tensor_copy(out=ot[:, :uw], in_=ps[:, :uw])
                eng.dma_start(out=out[:, RW * j0 : RW * j0 + uw],
                              in_=ot[:, :uw])
    if not nc.is_finalized():
        nc.finalize()
    return nc


def _decode(cfg, results, decode_quads):
    G, B, BATCH = cfg.G, cfg.B, cfg.BATCH
    out = np.empty((BATCH, G * B), dtype=np.float32)
    descale = 1.0 / (cfg.XS * cfg.WS)
    # batch-major result: quad j of core c sits at columns [RW*j : +RW],
    # laid out as QG genes x 8 outs.
    src_cols = np.empty(cfg.GPC * B, dtype=np.int64)   # per-core res columns
    dst_cols = np.empty(cfg.GPC * B, dtype=np.int64)   # global out columns
    for c in range(cfg.NCORES):
        res = np.asarray(results[c]["out"], dtype=np.float32)
        p = 0
        for j in range(cfg.NQ):
            f0 = cfg.QG * B * j
            for r, g in enumerate(decode_quads[c][j]):
                src_cols[p : p + B] = f0 + r * B + np.arange(B)
                dst_cols[p : p + B] = g * B + np.arange(B)
                p += B
        out[:, dst_cols] = res[:, src_cols]
    return out * descale


def _run(cfg, x, w, block_in, block_out, trace=False):
    in_maps, w_sched, decode_quads = _pack_host(cfg, x, w, block_in, block_out)
    nc = _build_nc(cfg, w_sched)
    r = run_bass_kernel_spmd(nc, in_maps, core_ids=list(range(cfg.NCORES)),
                             trace=trace)
    out = _decode(cfg, r.results, decode_quads)
    out += np.asarray(x, dtype=np.float32)      # exact residual on host
    return out, r


def kernel(x, w, block_in, block_out):
    cfg = Cfg()
    out, _ = _run(cfg, x, w, block_in, block_out, trace=False)
    return out



# revision 20
# speedup vs baseline: 1.0087x; 1.0087x over previous
"""Trainium2 Bass kernel for nn_LinearPPI (block-sparse gene-gene message passing).

Computation (reference):
    out[b, 8*g_out + o] = sum_{n: block_out[n]=g_out} sum_i x[b, 8*block_in[n] + i] * w[n, i, o]
    out += x   (residual)

Strategy:
  - Blocks sorted by destination gene; destination genes sharded over 8 cores
    (edge/expert parallel, no collectives needed).
  - Per core, genes are packed into "quads" of QG (default 2) genes.  A quad
    owns a [QG*8, 128] region of a PSUM bank (QG genes x 8 outs, 128 batch),
    laid out transposed (out^T).  16 quads fill one PSUM bank tile [128, 512].
  - Work is a stream of "windows": 16 x-slabs (one slab = 8 rows of x^T for
    one source gene = [8, 128]) stacked to a [128, 128] STATIONARY lhsT, and
    a matching scattered weight tile [128, QG*8] as the MOVING rhs.  One
    matmul per window:
        psum[0:128, f0:f0+QG*8] (+)= x_win.T @ w_win   (K=128, M=128, N=QG*8)
    The output is batch-major [128, QG*8] so the moving free dim is only
    QG*8=16 rows -> ~16 PE cycles per window instead of 128 (the batch dim
    rides in the stationary operand).  PSUM banks are fully dense: 32 quads
    per [128, 512] bank, 8 banks hold the core's whole output.
    PSUM per-element has_written bits turn the writes into a correct
    segment-sum; each quad's first matmul uses start=True (the bit clear
    only touches regions that are either finished or not yet started,
    and quads in one bank execute back-to-back on the PE).
  - The whole stream (x slabs + scattered weights) is fp8 e3m4: x scaled by
    XS=2, w by WS=32 so both sit in e3m4's normal range.  PSUM accumulates in
    fp32 at scale XS*WS; the host descales.  e3m4's 4 mantissa bits keep the
    end-to-end relative error ~1.2e-2 (measured), under the 2e-2 gate, while
    halving both DMA bytes and leaving PE at 1 cycle/row.
  - The residual (+x) is applied on the host in fp32 (exact), which also
    drops the 4000 virtual identity blocks from the stream.
  - The x-slab gather is done on the host (indices are known at trace time),
    producing a sequential HBM stream -> all device DMAs are large and
    contiguous (memory-bound regime).
  - The per-core window schedule is made identical across cores (rank-sorted
    window-count maxima + zero-padding) so a single SPMD program serves all
    8 cores; per-core variation lives only in the streamed data.
  - Output is slot-ordered out^T; the host inverse-permutes, transposes and
    concatenates shards.  No all-reduce: destination sharding makes each
    core's output disjoint.
"""

import math
import ml_dtypes
import numpy as np

import concourse.bass as bass
import concourse.bacc as bacc
import concourse.mybir as mybir
from concourse.tile import TileContext
from concourse.bass_utils import run_bass_kernel_spmd


class Cfg:
    def __init__(self, G=4000, B=8, BATCH=128, NCORES=8, chunk=24, qg=2,
                 xs=2.0, ws=32.0):
        assert G % NCORES == 0
        self.G, self.B, self.BATCH, self.NCORES = G, B, BATCH, NCORES
        self.GPC = G // NCORES            # genes per core
        self.QG = qg                      # genes per quad (M = QG*B)
        assert self.GPC % self.QG == 0
        self.NQ = self.GPC // self.QG     # quads per core
        self.UQ = 16                      # quads per drain unit (half a bank)
        self.UW = self.UQ * self.QG * B   # unit width in psum columns (256)
        self.NU = math.ceil(self.NQ / self.UQ)
        self.SLOTS = 16                   # slabs per window (K = 128)
        self.CH = chunk                   # windows per DMA chunk
        self.XS, self.WS = xs, ws         # fp8 pre-scales for x and w
        self.FP8_MAX = 15.5               # e3m4 saturation
        self.stream_np = ml_dtypes.float8_e3m4
        self.stream_dt = mybir.dt.float8e3
        self.out_dt = mybir.dt.float16


def _pack_host(cfg, x, w, block_in, block_out):
    """Sort/shard/pad on the host. Returns (in_maps, w_sched, decode_quads)."""
    G, B, BATCH, NC = cfg.G, cfg.B, cfg.BATCH, cfg.NCORES

    # Residual is applied on the host; stream only the real blocks.
    src = np.asarray(block_in, dtype=np.int64)
    dst = np.asarray(block_out, dtype=np.int64)
    fmax = cfg.FP8_MAX

    order = np.argsort(dst, kind="stable")
    src_s = src[order]
    w_s = np.clip(np.asarray(w, dtype=np.float32)[order] * cfg.WS,
                  -fmax, fmax).astype(cfg.stream_np)
    w_s = np.ascontiguousarray(w_s)
    counts = np.bincount(dst, minlength=G)
    starts = np.zeros(G + 1, dtype=np.int64)
    np.cumsum(counts, out=starts[1:])

    # x^T slabs: xslab[g] = x[:, 8g:8g+8].T  -> [G, 8, BATCH]
    xslab = np.clip(np.asarray(x, dtype=np.float32).T * cfg.XS,
                    -fmax, fmax).reshape(G, B, BATCH).astype(cfg.stream_np)
    xslab = np.ascontiguousarray(xslab)

    # --- balanced gene->core assignment (snake over count-sorted genes) ---
    order_g = np.argsort(-counts, kind="stable")
    core_of = np.empty(G, dtype=np.int64)
    for r in range(0, G, 2 * NC):
        blk = order_g[r : r + 2 * NC]
        pat = list(range(NC)) + list(range(NC - 1, -1, -1))
        for i, g in enumerate(blk):
            core_of[g] = pat[i]

    # --- per-core quad packing: target sums that are multiples of SLOTS ---
    per_core = []
    for c in range(NC):
        genes = np.where(core_of == c)[0]  # this core's genes
        pool = sorted(genes.tolist(), key=lambda g: -counts[g])
        quads = []
        for _ in range(cfg.NQ):
            q = [pool.pop(0)]                       # largest remaining
            while pool and len(q) < cfg.QG - 1:     # middle picks: big/small mix
                q.append(pool.pop(0) if len(q) % 2 else pool.pop(-1))
            if pool and len(q) < cfg.QG:
                s3 = sum(int(counts[g]) for g in q)
                # last pick: minimize padding to the next multiple of SLOTS
                best_i = min(range(len(pool)),
                             key=lambda i: (-(s3 + int(counts[pool[i]])))
                             % cfg.SLOTS)
                q.append(pool.pop(best_i))
            q.sort()
            quads.append(q)
        assert not pool
        q_slabs = np.array([sum(int(counts[g]) for g in q) for q in quads])
        q_wins = np.ceil(q_slabs / cfg.SLOTS).astype(np.int64)
        q_wins = np.maximum(q_wins, 1)
        rank = np.argsort(-q_wins, kind="stable")
        per_core.append(([quads[j] for j in rank], q_wins[rank]))

    # common schedule: per rank, max window count over cores
    w_sched = np.max(np.stack([pc[1] for pc in per_core]), axis=0)
    cum_w = np.zeros(cfg.NQ + 1, dtype=np.int64)
    np.cumsum(w_sched, out=cum_w[1:])
    w_tot = int(cum_w[-1])

    # --- build per-core streams -------------------------------------------
    in_maps = []
    decode_quads = []
    for c in range(NC):
        quads_r, _ = per_core[c]
        slab_gene = np.full(w_tot * cfg.SLOTS, -1, dtype=np.int64)
        blk_ids, blk_pos, blk_rel = [], [], []
        for j in range(cfg.NQ):
            base = cum_w[j] * cfg.SLOTS
            p = 0
            for r, g in enumerate(quads_r[j]):
                s0, n = int(starts[g]), int(counts[g])
                ids = np.arange(s0, s0 + n)
                blk_ids.append(ids)
                blk_pos.append(base + p + np.arange(n))
                blk_rel.append(np.full(n, r, dtype=np.int64))
                p += n
            assert p <= int(w_sched[j]) * cfg.SLOTS
        blk_ids = np.concatenate(blk_ids)
        blk_pos = np.concatenate(blk_pos)
        blk_rel = np.concatenate(blk_rel)
        slab_gene[blk_pos] = src_s[blk_ids]

        # x slabs: [W, 128, BATCH]
        xg = np.zeros((w_tot * cfg.SLOTS, B, BATCH), dtype=cfg.stream_np)
        m = slab_gene >= 0
        xg[m] = xslab[slab_gene[m]]
        xg = xg.reshape(w_tot, cfg.SLOTS * B, BATCH)

        # scattered weights: [W, 128, 32]
        wg5 = np.zeros((w_tot, cfg.SLOTS, B, cfg.QG, B), dtype=cfg.stream_np)
        wg5[blk_pos // cfg.SLOTS, blk_pos % cfg.SLOTS, :, blk_rel, :] = w_s[blk_ids]
        wg = wg5.reshape(w_tot, cfg.SLOTS * B, cfg.QG * B)

        # combined stream, chunk-major: each chunk of CH windows is one
        # contiguous [128, CH*PW] DRAM block -> every DMA is a single
        # linear ~1MB read.
        st = np.concatenate([xg, wg], axis=2)          # [W, 128, PW]
        PW = BATCH + cfg.QG * B
        n_chunks = -(-w_tot // cfg.CH)
        pad = n_chunks * cfg.CH - w_tot
        if pad:
            st = np.concatenate(
                [st, np.zeros((pad, cfg.SLOTS * B, PW), dtype=cfg.stream_np)])
        st = np.ascontiguousarray(
            st.reshape(n_chunks, cfg.CH, cfg.SLOTS * B, PW)
            .transpose(0, 2, 1, 3)).reshape(n_chunks * cfg.SLOTS * B, cfg.CH * PW)

        in_maps.append({"st": st})
        decode_quads.append(quads_r)

    return in_maps, w_sched, decode_quads


def _build_nc(cfg, w_sched):
    """Trace the (core-uniform) Bass program."""
    w_tot = int(np.sum(w_sched))
    PW = cfg.BATCH + cfg.QG * cfg.B   # stream width per window
    n_chunks = -(-w_tot // cfg.CH)
    nc = bacc.Bacc("TRN2")
    st = nc.dram_tensor("st", [n_chunks * 128, cfg.CH * PW], cfg.stream_dt,
                        kind="ExternalInput")
    out = nc.dram_tensor("out", [128, cfg.NQ * cfg.QG * cfg.B], cfg.out_dt,
                         kind="ExternalOutput")

    cum_w = np.zeros(cfg.NQ + 1, dtype=np.int64)
    np.cumsum(w_sched, out=cum_w[1:])
    CH = cfg.CH
    NW = cfg.BATCH            # rhs free width per window (128)

    with TileContext(nc) as tc:
        with (
            tc.tile_pool(name="stp", bufs=4) as stp,
            tc.tile_pool(name="psp", bufs=3, space="PSUM") as psp,
            tc.tile_pool(name="outp", bufs=2) as outp,
        ):
            RW = cfg.QG * cfg.B       # psum region width per quad
            st_t = None
            # Drain units: each owns its own psum tile, so unit copies never
            # share a tile with pending matmuls (no WAR stall).  Unit list is
            # (quad_lo, quad_hi, dma_engine); the final unit is a 2-quad
            # sliver drained via SP (cheapest HWDGE, idle at the end).
            units = []
            for u in range(cfg.NU):
                j0, j1 = u * cfg.UQ, min((u + 1) * cfg.UQ, cfg.NQ)
                if u == cfg.NU - 1 and j1 - j0 > 2:
                    units.append((j0, j1 - 2, nc.scalar))
                    units.append((j1 - 2, j1, nc.sync))
                else:
                    units.append((j0, j1, nc.scalar if u < cfg.NU - 1
                                  else nc.sync))
            for j0, j1, eng in units:
                uw = RW * (j1 - j0)
                ps = psp.tile([128, uw], mybir.dt.float32)
                ot = outp.tile([128, uw], cfg.out_dt)
                for j in range(j0, j1):
                    f0 = RW * (j - j0)
                    t_first = int(cum_w[j])
                    for t in range(int(cum_w[j]), int(cum_w[j + 1])):
                        if t % CH == 0:
                            c = t // CH
                            st_t = stp.tile([128, CH * PW], cfg.stream_dt)
                            nc.sync.dma_start(
                                out=st_t[:, :],
                                in_=st[c * 128 : (c + 1) * 128, :])
                        k = t % CH
                        nc.tensor.matmul(
                            ps[:, f0 : f0 + RW],
                            st_t[:, k * PW : k * PW + NW],
                            st_t[:, k * PW + NW : (k + 1) * PW],
                            start=(t == t_first),
                            stop=(t == int(cum_w[j + 1]) - 1),
                            tile_position=(0, 0),
                        )
                nc.vector.tensor_copy(out=ot[:, :uw], in_=ps[:, :uw])
                eng.dma_start(out=out[:, RW * j0 : RW * j0 + uw],
                              in_=ot[:, :uw])
    if not nc.is_finalized():
        nc.finalize()
    return nc


def _decode(cfg, results, decode_quads):
    G, B, BATCH = cfg.G, cfg.B, cfg.BATCH
    out = np.empty((BATCH, G * B), dtype=np.float32)
    descale = 1.0 / (cfg.XS * cfg.WS)
    # batch-major result: quad j of core c sits at columns [RW*j : +RW],
    # laid out as QG genes x 8 outs.
    src_cols = np.empty(cfg.GPC * B, dtype=np.int64)   # per-core res columns
    dst_cols = np.empty(cfg.GPC * B, dtype=np.int64)   # global out columns
    for c in range(cfg.NCORES):
        res = np.asarray(results[c]["out"], dtype=np.float32)
        p = 0
        for j in range(cfg.NQ):
            f0 = cfg.QG * B * j
            for r, g in enumerate(decode_quads[c][j]):
                src_cols[p : p + B] = f0 + r * B + np.arange(B)
                dst_cols[p : p + B] = g * B + np.arange(B)
                p += B
        out[:, dst_cols] = res[:, src_cols]
    return out * descale


def _run(cfg, x, w, block_in, block_out, trace=False):
    in_maps, w_sched, decode_quads = _pack_host(cfg, x, w, block_in, block_out)
    nc = _build_nc(cfg, w_sched)
    r = run_bass_kernel_spmd(nc, in_maps, core_ids=list(range(cfg.NCORES)),
                             trace=trace)
    out = _decode(cfg, r.results, decode_quads)
    out += np.asarray(x, dtype=np.float32)      # exact residual on host
    return out, r


def kernel(x, w, block_in, block_out):
    cfg = Cfg()
    out, _ = _run(cfg, x, w, block_in, block_out, trace=False)
    return out

